# revision 6
# baseline (speedup 1.0000x reference)
"""ConvShapeletFilter kernel for Trainium2 (8 NeuronCores, data-parallel).

Math: reference computes, per batch row b and shapelet k,
    corr[b,n,k] = <x_win[b,n]-mean(x_win[b,n]), s[k]-mean(s[k])>
Since (s[k]-mean(s[k])) sums to zero over l, the window-mean term drops:
    corr[b,n,k] = sum_l x[b,n+l] * s_norm[k,l]
i.e. a plain cross-correlation with the mean-removed shapelet bank.
Outputs per (b,k): top-1, mean(top-5), top-2, relu(top1-top2) over n.

Device mapping (per core, 32 of 256 batch rows), mode "bf16" (default):
  - s_norm^T prepared host-side (tiny [128,128] op) and shipped bf16.
  - One Hankel/im2col tile per row, H[l, c] = x[b, l + c], [128, 4096]
    bf16, DMA'd straight from HBM with an overlapping access pattern.
    The AP's outermost dim is the 128-partition dim: the hardware DGE
    round-robins descriptors over the 16 DMA engines by outer-dim
    index, so this spreads the ~0.5 MB twice-per-row traffic evenly
    (a small outer dim pins everything to 2 engines - 3x slower
    end-to-end). Rows alternate between the SP and Activation HWDGE
    queues.
  - Per row, 2 PSUM spans of 2048 windows; 4 matmuls per span
    (full L=128 contraction, bf16 operands, fp32 PSUM).
  - DVE InstMax (sorted top-8 per partition) once per span -> CAND
    slots. DVE does nothing else in steady state: the per-row top-8
    merge + finalization is deferred and batched.
  - Batched finalize: with per-row sorted span lists a, b, the k-th
    largest of the union is max_{i+j=k-1} min(a_i, b_j). Computed for
    k=1..5 over all 32 rows at once via ~31 strided [128,32]
    elementwise ops, giving p1, p2, mean(top5), dominance directly.
    p1 >= p2 always, so relu(p1-p2) is a plain subtract.
  - One PE transpose of the [K, 128] result + 4 DMAs write y[32, 512].

Mode "f32r" (fallback): fp32 data bitcast float32r, G=64 filter split
with PSUM accumulation to halve DMA traffic; per-row finalize.
"""

import os
import sys

for _p in ("/opt/trn_rl_repo", os.path.expanduser("~/.axon_site/_ro/trn_rl_repo")):
    if os.path.isdir(_p) and _p not in sys.path:
        sys.path.insert(0, _p)

import numpy as np

MODE = os.environ.get("SHAPELET_MODE", "bf16")        # bf16 | f32r
MERGE_ENGINE = os.environ.get("SHAPELET_MERGE", "vector")  # vector | gpsimd

B, T = 256, 4096
L = 128
K = 128
K_TOP = 5
N = T - L + 1          # 3969 sliding windows
N_CORES = 8
ROWS = B // N_CORES    # 32 batch rows per core
WBLK = 512             # windows per matmul (1 PSUM bank fp32)
HALF = 2048            # windows per PSUM span (4 banks)
OUT_COLS = 4 * K       # p1 | p_mean | p2 | dominance
G = 64                 # f32r mode: filter-chunk size
TPAD = T + 2 * G       # padded x row length (4224)
HW = T                 # bf16 hankel tile width (cols 0..4095)


def _split_excess_waits(nc, mybir, max_waits=1):
    """Walrus CoreV3 codegen rejects >1 sync-wait on several instruction
    classes (CTRL/Drain, S3_LW/Matmult, ...). Hoist excess waits onto
    same-engine NoOps placed just before the offender."""
    for fn in nc.m.functions:
        for bb in fn.blocks:
            insts = bb.instructions
            i = 0
            while i < len(insts):
                inst = insts[i]
                si = inst.sync_info
                if (si is not None and si.on_wait
                        and len(si.on_wait) > max_waits):
                    waits = list(si.on_wait)
                    si.on_wait = waits[:max_waits]
                    for cs in range(max_waits, len(waits), max_waits):
                        chunk = waits[cs:cs + max_waits]
                        d = nc.sync.nop(nofuse=True)
                        cur = nc.cur_bb.bb.instructions
                        assert cur[-1] is d.ins
                        cur.pop()
                        d.ins.engine = inst.engine
                        d.ins.sync_info = mybir.SyncInfo(on_wait=chunk, on_update=[])
                        insts.insert(i, d.ins)
                        i += 1
                i += 1


def build_program(mode=MODE):
    import concourse.bass as bass
    import concourse.mybir as mybir
    from concourse.masks import make_identity
    from concourse.tile import TileContext

    f32 = mybir.dt.float32
    io_dt = {"bf16": mybir.dt.bfloat16, "f32r": mybir.dt.float32r}[mode]
    snt_shape = [L, K] if mode == "bf16" else [128, 2 * K]

    nc = bass.Bass()
    x = nc.declare_dram_parameter("x", [ROWS, TPAD], io_dt, isOutput=False)
    snt_in = nc.declare_dram_parameter("snt", snt_shape, io_dt, isOutput=False)
    y = nc.declare_dram_parameter("y", [ROWS, OUT_COLS], f32, isOutput=True)

    def hankel_ap(b, col0, g_rows, width, n_chunk, chunk_step):
        """AP over x: dims (chunk, l, c) -> x[b, col0 + chunk*chunk_step + l + c]."""
        ap = x[b:b + 1, 0:width].copy()
        dims = [[1, g_rows], [1, width]]
        if n_chunk > 1:
            dims = [[chunk_step, n_chunk]] + dims
        ap.ap = mybir.VecI64Pair(dims)
        ap.offset = b * TPAD + col0
        return ap

    halves = [(0, HALF), (HALF, N - HALF)]   # (n0, n_windows)

    with TileContext(nc) as tc:
        with (
            tc.tile_pool(name="const", bufs=1) as const_pool,
            tc.tile_pool(name="hank", bufs=3) as hank_pool,
            tc.tile_pool(name="mrg", bufs=1) as mrg_pool,
        ):
            snt = const_pool.tile(snt_shape, io_dt)
            nc.sync.dma_start(out=snt[:, :], in_=snt_in[:, :])
            ident = const_pool.tile([128, 128], f32)
            make_identity(nc, ident[:, :])
            # Result accumulator R[k, m*32 + b], m in (p1, p_mean, p2, dom).
            R = const_pool.tile([K, 128], f32)
            # Per-row sorted top-8 candidates: row b spans cols
            # [16b, 16b+16); span A top-8 first, span B top-8 second.
            CAND = const_pool.tile([K, 16 * ROWS], f32)

            with tc.tile_pool(name="psum", bufs=2, space="PSUM") as psum_pool:
                for b in range(ROWS):
                    eng = nc.sync if b % 2 == 0 else nc.scalar
                    if mode == "bf16":
                        h = hank_pool.tile([128, HW], io_dt, tag="hank")
                        if b < 2:
                            # First rows: split across both HWDGE queues so
                            # row 0's descriptors get all 16 DMA engines at
                            # once - later rows' prefetch otherwise delays
                            # row 0 and with it the whole pipeline start.
                            nc.sync.dma_start(
                                out=h[0:64, :],
                                in_=hankel_ap(b, 0, 64, HW, 1, 0))
                            sc = hankel_ap(b, 0, 64, HW, 1, 0)
                            sc.offset += 64
                            nc.scalar.dma_start(out=h[64:128, :], in_=sc)
                        else:
                            eng.dma_start(out=h[:, :],
                                          in_=hankel_ap(b, 0, 128, HW, 1, 0))
                    else:
                        w = HALF + G
                        h = hank_pool.tile([128, w], io_dt, tag="hank")
                        for ci, ceng in ((0, nc.sync), (1, nc.scalar)):
                            ceng.dma_start(
                                out=h[ci * G:(ci + 1) * G, :],
                                in_=hankel_ap(b, ci * HALF, G, w, 1, 0))

                    for hi, (n0, nw) in enumerate(halves):
                        ps = psum_pool.tile([K, HALF], f32, tag="psum")
                        for j in range(0, HALF, WBLK):
                            if mode == "bf16":
                                nc.tensor.matmul(
                                    ps[:, j:j + WBLK], snt[:, :],
                                    h[:, n0 + j:n0 + j + WBLK],
                                    start=True, stop=True)
                            else:
                                r0 = hi * G
                                nc.tensor.matmul(
                                    ps[:, j:j + WBLK],
                                    snt[r0:r0 + G, 0:K],
                                    h[r0:r0 + G, j:j + WBLK],
                                    start=True, stop=False)
                                nc.tensor.matmul(
                                    ps[:, j:j + WBLK],
                                    snt[r0:r0 + G, K:2 * K],
                                    h[r0:r0 + G, j + G:j + G + WBLK],
                                    start=False, stop=True)
                        c0 = 16 * b + 8 * hi
                        nc.vector.max(out=CAND[:, c0:c0 + 8], in_=ps[:, :nw])

            # ---- batched finalize over all 32 rows ----------------------
            # a_i / b_i: [K, 32] strided views, element (k, b) = i-th
            # largest of row b's span A / span B.
            e = nc.gpsimd if MERGE_ENGINE == "gpsimd" else nc.vector
            mmin = mybir.AluOpType.min

            def a(i):
                return CAND[:, i::16]

            def bb(i):
                return CAND[:, 8 + i::16]

            _tmp_n = [0]

            def tmp():
                # unique tag per temp: each gets its own SBUF allocation
                # (same-tag tiles in a pool share a buffer ring)
                _tmp_n[0] += 1
                nm = f"mrgt{_tmp_n[0]}"
                return mrg_pool.tile([K, ROWS], f32, tag=nm, name=nm)

            p1 = R[:, 0:ROWS]
            pm = R[:, ROWS:2 * ROWS]
            p2 = R[:, 2 * ROWS:3 * ROWS]
            dom = R[:, 3 * ROWS:4 * ROWS]

            # t1
            e.tensor_max(p1, a(0), bb(0))
            # t2 = max(a1, b1, min(a0,b0))
            m01 = tmp()
            e.tensor_tensor(m01[:, :], a(0), bb(0), mmin)
            u = tmp()
            e.tensor_max(u[:, :], a(1), bb(1))
            e.tensor_max(p2, u[:, :], m01[:, :])
            # dominance = p1 - p2 (p1 >= p2 always)
            e.tensor_sub(dom, p1, p2)
            # t3 = max(a2, b2, min(a0,b1), min(a1,b0))
            v1, v2, v3 = tmp(), tmp(), tmp()
            e.tensor_max(v1[:, :], a(2), bb(2))
            e.tensor_tensor(v2[:, :], a(0), bb(1), mmin)
            e.tensor_tensor(v3[:, :], a(1), bb(0), mmin)
            e.tensor_max(v2[:, :], v2[:, :], v3[:, :])
            t3 = tmp()
            e.tensor_max(t3[:, :], v1[:, :], v2[:, :])
            # t4 = max(a3, b3, min(a0,b2), min(a1,b1), min(a2,b0))
            w1, w2, w3 = tmp(), tmp(), tmp()
            e.tensor_max(w1[:, :], a(3), bb(3))
            e.tensor_tensor(w2[:, :], a(0), bb(2), mmin)
            e.tensor_tensor(w3[:, :], a(1), bb(1), mmin)
            e.tensor_max(w2[:, :], w2[:, :], w3[:, :])
            e.tensor_tensor(w3[:, :], a(2), bb(0), mmin)
            e.tensor_max(w1[:, :], w1[:, :], w2[:, :])
            t4 = tmp()
            e.tensor_max(t4[:, :], w1[:, :], w3[:, :])
            # t5 = max(a4, b4, min(a0,b3), min(a1,b2), min(a2,b1), min(a3,b0))
            z1, z2, z3 = tmp(), tmp(), tmp()
            e.tensor_max(z1[:, :], a(4), bb(4))
            e.tensor_tensor(z2[:, :], a(0), bb(3), mmin)
            e.tensor_tensor(z3[:, :], a(1), bb(2), mmin)
            e.tensor_max(z2[:, :], z2[:, :], z3[:, :])
            e.tensor_tensor(z3[:, :], a(2), bb(1), mmin)
            e.tensor_max(z1[:, :], z1[:, :], z2[:, :])
            e.tensor_tensor(z2[:, :], a(3), bb(0), mmin)
            e.tensor_max(z3[:, :], z3[:, :], z2[:, :])
            t5 = tmp()
            e.tensor_max(t5[:, :], z1[:, :], z3[:, :])
            # p_mean = (t1 + t2 + t3 + t4 + t5) / 5
            s = tmp()
            e.tensor_add(s[:, :], p1, p2)
            e.tensor_add(s[:, :], s[:, :], t3[:, :])
            e.tensor_add(s[:, :], s[:, :], t4[:, :])
            e.tensor_add(s[:, :], s[:, :], t5[:, :])
            e.tensor_scalar_mul(pm, s[:, :], 1.0 / K_TOP)

            # Transpose R -> TR[m*32+b, k]; write y[b, m*128+k].
            with tc.tile_pool(name="tpsum", bufs=1, space="PSUM") as tpsum_pool:
                tr_ps = tpsum_pool.tile([128, 128], f32)
                nc.tensor.transpose(tr_ps[:, :], R[:, :], ident[:, :])
                tr = const_pool.tile([128, 128], f32)
                nc.scalar.copy(tr[:, :], tr_ps[:, :])
                for m in range(4):
                    nc.sync.dma_start(out=y[:, m * K:(m + 1) * K],
                                      in_=tr[m * ROWS:(m + 1) * ROWS, :])

    _split_excess_waits(nc, mybir)
    return nc


_CACHED = {}


def _get_program(mode=MODE):
    if mode not in _CACHED:
        _CACHED[mode] = build_program(mode)
    return _CACHED[mode]


def _prep_inputs(x, shapelets, mode=MODE):
    x = np.ascontiguousarray(x, dtype=np.float32)
    s = np.asarray(shapelets, dtype=np.float32)
    snt = np.ascontiguousarray((s - s.mean(axis=1, keepdims=True)).T)
    x = np.pad(x, ((0, 0), (0, TPAD - T)))
    if mode == "bf16":
        import ml_dtypes
        x = x.astype(ml_dtypes.bfloat16)
        snt = snt.astype(ml_dtypes.bfloat16)
    else:
        # [128, 2K]: G-row chunks side by side, tiled over both halves.
        snt = np.tile(np.concatenate([snt[0:G], snt[G:2 * G]], axis=1), (2, 1))
        snt = np.ascontiguousarray(snt)
    return x, snt


def run_sharded(x, shapelets, mode=MODE, trace=False, **kw):
    from concourse.bass_utils import run_bass_kernel_spmd

    nc = _get_program(mode)
    xp, snt = _prep_inputs(x, shapelets, mode)
    in_maps = [
        {"x": xp[c * ROWS:(c + 1) * ROWS], "snt": snt}
        for c in range(N_CORES)
    ]
    res = run_bass_kernel_spmd(nc, in_maps, list(range(N_CORES)), trace=trace, **kw)
    out = np.concatenate([res.results[c]["y"] for c in range(N_CORES)], axis=0)
    return out, res


def kernel(x, shapelets):
    out, _ = run_sharded(x, shapelets)
    return out


# revision 7
# speedup vs baseline: 1.1772x; 1.1772x over previous
"""ConvShapeletFilter kernel for Trainium2 (8 NeuronCores, data-parallel).

Math: reference computes, per batch row b and shapelet k,
    corr[b,n,k] = <x_win[b,n]-mean(x_win[b,n]), s[k]-mean(s[k])>
Since (s[k]-mean(s[k])) sums to zero over l, the window-mean term drops:
    corr[b,n,k] = sum_l x[b,n+l] * s_norm[k,l]
i.e. a plain cross-correlation with the mean-removed shapelet bank.
Outputs per (b,k): top-1, mean(top-5), top-2, relu(top1-top2) over n.

Device mapping (per core, 32 of 256 batch rows), mode "bf16" (default):
  - s_norm^T prepared host-side (tiny [128,128] op) and shipped bf16.
  - One Hankel/im2col tile per row, H[l, c] = x[b, l + c], [128, 4096]
    bf16, DMA'd straight from HBM with an overlapping access pattern.
    The AP's outermost dim is the 128-partition dim: the hardware DGE
    round-robins descriptors over the 16 DMA engines by outer-dim
    index, so this spreads the ~0.5 MB twice-per-row traffic evenly
    (a small outer dim pins everything to 2 engines - 3x slower
    end-to-end). Rows alternate between the SP and Activation HWDGE
    queues.
  - Per row, 2 PSUM spans of 2048 windows; 4 matmuls per span
    (full L=128 contraction, bf16 operands, fp32 PSUM).
  - DVE InstMax (sorted top-8 per partition) once per span -> CAND
    slots. DVE does nothing else in steady state: the per-row top-8
    merge + finalization is deferred and batched.
  - Batched finalize: with per-row sorted span lists a, b, the k-th
    largest of the union is max_{i+j=k-1} min(a_i, b_j). Computed for
    k=1..5 over all 32 rows at once via ~31 strided [128,32]
    elementwise ops, giving p1, p2, mean(top5), dominance directly.
    p1 >= p2 always, so relu(p1-p2) is a plain subtract.
  - One PE transpose of the [K, 128] result + 4 DMAs write y[32, 512].

Mode "f32r" (fallback): fp32 data bitcast float32r, G=64 filter split
with PSUM accumulation to halve DMA traffic; per-row finalize.
"""

import os
import sys

for _p in ("/opt/trn_rl_repo", os.path.expanduser("~/.axon_site/_ro/trn_rl_repo")):
    if os.path.isdir(_p) and _p not in sys.path:
        sys.path.insert(0, _p)

import numpy as np

MODE = os.environ.get("SHAPELET_MODE", "bf16")        # bf16 | f32r
MERGE_ENGINE = os.environ.get("SHAPELET_MERGE", "vector")  # vector | gpsimd

B, T = 256, 4096
L = 128
K = 128
K_TOP = 5
N = T - L + 1          # 3969 sliding windows
N_CORES = 8
ROWS = B // N_CORES    # 32 batch rows per core
WBLK = 512             # windows per matmul (1 PSUM bank fp32)
HALF = 2048            # windows per PSUM span (4 banks)
OUT_COLS = 4 * K       # p1 | p_mean | p2 | dominance
G = 64                 # f32r mode: filter-chunk size
TPAD = T + 2 * G       # padded x row length (4224)
HW = T                 # bf16 hankel tile width (cols 0..4095)


def _split_excess_waits(nc, mybir, max_waits=1):
    """Walrus CoreV3 codegen rejects >1 sync-wait on several instruction
    classes (CTRL/Drain, S3_LW/Matmult, ...). Hoist excess waits onto
    same-engine NoOps placed just before the offender."""
    for fn in nc.m.functions:
        for bb in fn.blocks:
            insts = bb.instructions
            i = 0
            while i < len(insts):
                inst = insts[i]
                si = inst.sync_info
                if (si is not None and si.on_wait
                        and len(si.on_wait) > max_waits):
                    waits = list(si.on_wait)
                    si.on_wait = waits[:max_waits]
                    for cs in range(max_waits, len(waits), max_waits):
                        chunk = waits[cs:cs + max_waits]
                        d = nc.sync.nop(nofuse=True)
                        cur = nc.cur_bb.bb.instructions
                        assert cur[-1] is d.ins
                        cur.pop()
                        d.ins.engine = inst.engine
                        d.ins.sync_info = mybir.SyncInfo(on_wait=chunk, on_update=[])
                        insts.insert(i, d.ins)
                        i += 1
                i += 1


def build_program(mode=MODE):
    import concourse.bass as bass
    import concourse.mybir as mybir
    from concourse.masks import make_identity
    from concourse.tile import TileContext

    f32 = mybir.dt.float32
    io_dt = {"bf16": mybir.dt.bfloat16, "f32r": mybir.dt.float32r}[mode]
    snt_shape = [L, K] if mode == "bf16" else [128, 2 * K]

    nc = bass.Bass()
    x = nc.declare_dram_parameter("x", [ROWS, TPAD], io_dt, isOutput=False)
    snt_in = nc.declare_dram_parameter("snt", snt_shape, io_dt, isOutput=False)
    y = nc.declare_dram_parameter("y", [ROWS, OUT_COLS], f32, isOutput=True)

    def hankel_ap(b, col0, g_rows, width, n_chunk, chunk_step):
        """AP over x: dims (chunk, l, c) -> x[b, col0 + chunk*chunk_step + l + c]."""
        ap = x[b:b + 1, 0:width].copy()
        dims = [[1, g_rows], [1, width]]
        if n_chunk > 1:
            dims = [[chunk_step, n_chunk]] + dims
        ap.ap = mybir.VecI64Pair(dims)
        ap.offset = b * TPAD + col0
        return ap

    halves = [(0, HALF), (HALF, N - HALF)]   # (n0, n_windows)

    with TileContext(nc) as tc:
        with (
            tc.tile_pool(name="const", bufs=1) as const_pool,
            tc.tile_pool(name="hank", bufs=3) as hank_pool,
            tc.tile_pool(name="mrg", bufs=1) as mrg_pool,
        ):
            snt = const_pool.tile(snt_shape, io_dt)
            nc.sync.dma_start(out=snt[:, :], in_=snt_in[:, :])
            ident = const_pool.tile([128, 128], f32)
            make_identity(nc, ident[:, :])
            # Result accumulator R[k, m*32 + b], m in (p1, p_mean, p2, dom).
            R = const_pool.tile([K, 128], f32)
            # Per-row sorted top-8 candidates: row b spans cols
            # [16b, 16b+16); span A top-8 first, span B top-8 second.
            CAND = const_pool.tile([K, 16 * ROWS], f32)

            with tc.tile_pool(name="psum", bufs=2, space="PSUM") as psum_pool:
                for b in range(ROWS):
                    eng = nc.sync if b % 2 == 0 else nc.scalar
                    if mode == "bf16":
                        h = hank_pool.tile([128, HW], io_dt, tag="hank")
                        eng.dma_start(out=h[:, :],
                                      in_=hankel_ap(b, 0, 128, HW, 1, 0))
                    else:
                        w = HALF + G
                        h = hank_pool.tile([128, w], io_dt, tag="hank")
                        for ci, ceng in ((0, nc.sync), (1, nc.scalar)):
                            ceng.dma_start(
                                out=h[ci * G:(ci + 1) * G, :],
                                in_=hankel_ap(b, ci * HALF, G, w, 1, 0))

                    for hi, (n0, nw) in enumerate(halves):
                        ps = psum_pool.tile([K, HALF], f32, tag="psum")
                        for j in range(0, HALF, WBLK):
                            if mode == "bf16":
                                nc.tensor.matmul(
                                    ps[:, j:j + WBLK], snt[:, :],
                                    h[:, n0 + j:n0 + j + WBLK],
                                    start=True, stop=True)
                            else:
                                r0 = hi * G
                                nc.tensor.matmul(
                                    ps[:, j:j + WBLK],
                                    snt[r0:r0 + G, 0:K],
                                    h[r0:r0 + G, j:j + WBLK],
                                    start=True, stop=False)
                                nc.tensor.matmul(
                                    ps[:, j:j + WBLK],
                                    snt[r0:r0 + G, K:2 * K],
                                    h[r0:r0 + G, j + G:j + G + WBLK],
                                    start=False, stop=True)
                        c0 = 16 * b + 8 * hi
                        nc.vector.max(out=CAND[:, c0:c0 + 8], in_=ps[:, :nw])

            # ---- batched finalize over all 32 rows ----------------------
            # a_i / b_i: [K, 32] strided views, element (k, b) = i-th
            # largest of row b's span A / span B.
            e = nc.gpsimd if MERGE_ENGINE == "gpsimd" else nc.vector
            mmin = mybir.AluOpType.min

            def a(i):
                return CAND[:, i::16]

            def bb(i):
                return CAND[:, 8 + i::16]

            _tmp_n = [0]

            def tmp():
                # unique tag per temp: each gets its own SBUF allocation
                # (same-tag tiles in a pool share a buffer ring)
                _tmp_n[0] += 1
                nm = f"mrgt{_tmp_n[0]}"
                return mrg_pool.tile([K, ROWS], f32, tag=nm, name=nm)

            p1 = R[:, 0:ROWS]
            pm = R[:, ROWS:2 * ROWS]
            p2 = R[:, 2 * ROWS:3 * ROWS]
            dom = R[:, 3 * ROWS:4 * ROWS]

            # t1
            e.tensor_max(p1, a(0), bb(0))
            # t2 = max(a1, b1, min(a0,b0))
            m01 = tmp()
            e.tensor_tensor(m01[:, :], a(0), bb(0), mmin)
            u = tmp()
            e.tensor_max(u[:, :], a(1), bb(1))
            e.tensor_max(p2, u[:, :], m01[:, :])
            # dominance = p1 - p2 (p1 >= p2 always)
            e.tensor_sub(dom, p1, p2)
            # t3 = max(a2, b2, min(a0,b1), min(a1,b0))
            v1, v2, v3 = tmp(), tmp(), tmp()
            e.tensor_max(v1[:, :], a(2), bb(2))
            e.tensor_tensor(v2[:, :], a(0), bb(1), mmin)
            e.tensor_tensor(v3[:, :], a(1), bb(0), mmin)
            e.tensor_max(v2[:, :], v2[:, :], v3[:, :])
            t3 = tmp()
            e.tensor_max(t3[:, :], v1[:, :], v2[:, :])
            # t4 = max(a3, b3, min(a0,b2), min(a1,b1), min(a2,b0))
            w1, w2, w3 = tmp(), tmp(), tmp()
            e.tensor_max(w1[:, :], a(3), bb(3))
            e.tensor_tensor(w2[:, :], a(0), bb(2), mmin)
            e.tensor_tensor(w3[:, :], a(1), bb(1), mmin)
            e.tensor_max(w2[:, :], w2[:, :], w3[:, :])
            e.tensor_tensor(w3[:, :], a(2), bb(0), mmin)
            e.tensor_max(w1[:, :], w1[:, :], w2[:, :])
            t4 = tmp()
            e.tensor_max(t4[:, :], w1[:, :], w3[:, :])
            # t5 = max(a4, b4, min(a0,b3), min(a1,b2), min(a2,b1), min(a3,b0))
            z1, z2, z3 = tmp(), tmp(), tmp()
            e.tensor_max(z1[:, :], a(4), bb(4))
            e.tensor_tensor(z2[:, :], a(0), bb(3), mmin)
            e.tensor_tensor(z3[:, :], a(1), bb(2), mmin)
            e.tensor_max(z2[:, :], z2[:, :], z3[:, :])
            e.tensor_tensor(z3[:, :], a(2), bb(1), mmin)
            e.tensor_max(z1[:, :], z1[:, :], z2[:, :])
            e.tensor_tensor(z2[:, :], a(3), bb(0), mmin)
            e.tensor_max(z3[:, :], z3[:, :], z2[:, :])
            t5 = tmp()
            e.tensor_max(t5[:, :], z1[:, :], z3[:, :])
            # p_mean = (t1 + t2 + t3 + t4 + t5) / 5
            s = tmp()
            e.tensor_add(s[:, :], p1, p2)
            e.tensor_add(s[:, :], s[:, :], t3[:, :])
            e.tensor_add(s[:, :], s[:, :], t4[:, :])
            e.tensor_add(s[:, :], s[:, :], t5[:, :])
            e.tensor_scalar_mul(pm, s[:, :], 1.0 / K_TOP)

            # Transpose R -> TR[m*32+b, k]; write y[b, m*128+k].
            with tc.tile_pool(name="tpsum", bufs=1, space="PSUM") as tpsum_pool:
                tr_ps = tpsum_pool.tile([128, 128], f32)
                nc.tensor.transpose(tr_ps[:, :], R[:, :], ident[:, :])
                tr = const_pool.tile([128, 128], f32)
                nc.scalar.copy(tr[:, :], tr_ps[:, :])
                for m in range(4):
                    nc.sync.dma_start(out=y[:, m * K:(m + 1) * K],
                                      in_=tr[m * ROWS:(m + 1) * ROWS, :])

    _split_excess_waits(nc, mybir)
    return nc


_CACHED = {}


def _get_program(mode=MODE):
    if mode not in _CACHED:
        _CACHED[mode] = build_program(mode)
    return _CACHED[mode]


def _prep_inputs(x, shapelets, mode=MODE):
    x = np.ascontiguousarray(x, dtype=np.float32)
    s = np.asarray(shapelets, dtype=np.float32)
    snt = np.ascontiguousarray((s - s.mean(axis=1, keepdims=True)).T)
    x = np.pad(x, ((0, 0), (0, TPAD - T)))
    if mode == "bf16":
        import ml_dtypes
        x = x.astype(ml_dtypes.bfloat16)
        snt = snt.astype(ml_dtypes.bfloat16)
    else:
        # [128, 2K]: G-row chunks side by side, tiled over both halves.
        snt = np.tile(np.concatenate([snt[0:G], snt[G:2 * G]], axis=1), (2, 1))
        snt = np.ascontiguousarray(snt)
    return x, snt


def run_sharded(x, shapelets, mode=MODE, trace=False, **kw):
    from concourse.bass_utils import run_bass_kernel_spmd

    nc = _get_program(mode)
    xp, snt = _prep_inputs(x, shapelets, mode)
    in_maps = [
        {"x": xp[c * ROWS:(c + 1) * ROWS], "snt": snt}
        for c in range(N_CORES)
    ]
    res = run_bass_kernel_spmd(nc, in_maps, list(range(N_CORES)), trace=trace, **kw)
    out = np.concatenate([res.results[c]["y"] for c in range(N_CORES)], axis=0)
    return out, res


def kernel(x, shapelets):
    out, _ = run_sharded(x, shapelets)
    return out


# revision 8
# speedup vs baseline: 1.1962x; 1.0161x over previous
"""ConvShapeletFilter kernel for Trainium2 (8 NeuronCores, data-parallel).

Math: reference computes, per batch row b and shapelet k,
    corr[b,n,k] = <x_win[b,n]-mean(x_win[b,n]), s[k]-mean(s[k])>
Since (s[k]-mean(s[k])) sums to zero over l, the window-mean term drops:
    corr[b,n,k] = sum_l x[b,n+l] * s_norm[k,l]
i.e. a plain cross-correlation with the mean-removed shapelet bank.
Outputs per (b,k): top-1, mean(top-5), top-2, relu(top1-top2) over n.

Device mapping (per core, 32 of 256 batch rows), mode "bf16" (default):
  - s_norm^T prepared host-side (tiny [128,128] op) and shipped bf16.
  - One Hankel/im2col tile per row, H[l, c] = x[b, l + c], [128, 4096]
    bf16, DMA'd straight from HBM with an overlapping access pattern.
    The AP's outermost dim is the 128-partition dim: the hardware DGE
    round-robins descriptors over the 16 DMA engines by outer-dim
    index, so this spreads the ~0.5 MB twice-per-row traffic evenly
    (a small outer dim pins everything to 2 engines - 3x slower
    end-to-end). Rows alternate between the SP and Activation HWDGE
    queues.
  - Per row, 2 PSUM spans of 2048 windows; 4 matmuls per span
    (full L=128 contraction, bf16 operands, fp32 PSUM).
  - DVE InstMax (sorted top-8 per partition) once per span -> CAND
    slots. DVE does nothing else in steady state: the per-row top-8
    merge + finalization is deferred and batched.
  - Batched finalize: with per-row sorted span lists a, b, the k-th
    largest of the union is max_{i+j=k-1} min(a_i, b_j). Computed for
    k=1..5 over all 32 rows at once via ~31 strided [128,32]
    elementwise ops, giving p1, p2, mean(top5), dominance directly.
    p1 >= p2 always, so relu(p1-p2) is a plain subtract.
  - One PE transpose of the [K, 128] result + 4 DMAs write y[32, 512].

Mode "f32r" (fallback): fp32 data bitcast float32r, G=64 filter split
with PSUM accumulation to halve DMA traffic; per-row finalize.
"""

import os
import sys

for _p in ("/opt/trn_rl_repo", os.path.expanduser("~/.axon_site/_ro/trn_rl_repo")):
    if os.path.isdir(_p) and _p not in sys.path:
        sys.path.insert(0, _p)

import numpy as np

MODE = os.environ.get("SHAPELET_MODE", "bf16")        # bf16 | f32r
MERGE_ENGINE = os.environ.get("SHAPELET_MERGE", "vector")  # vector | gpsimd

B, T = 256, 4096
L = 128
K = 128
K_TOP = 5
N = T - L + 1          # 3969 sliding windows
N_CORES = 8
ROWS = B // N_CORES    # 32 batch rows per core
WBLK = 512             # windows per matmul (1 PSUM bank fp32)
HALF = 2048            # windows per PSUM span (4 banks)
OUT_COLS = 4 * K       # p1 | p_mean | p2 | dominance
G = 64                 # f32r mode: filter-chunk size
TPAD = T + 2 * G       # padded x row length (4224)
HW = T                 # bf16 hankel tile width (cols 0..4095)


def _split_excess_waits(nc, mybir, max_waits=1):
    """Walrus CoreV3 codegen rejects >1 sync-wait on several instruction
    classes (CTRL/Drain, S3_LW/Matmult, ...). Hoist excess waits onto
    same-engine NoOps placed just before the offender."""
    for fn in nc.m.functions:
        for bb in fn.blocks:
            insts = bb.instructions
            i = 0
            while i < len(insts):
                inst = insts[i]
                si = inst.sync_info
                if (si is not None and si.on_wait
                        and len(si.on_wait) > max_waits):
                    waits = list(si.on_wait)
                    si.on_wait = waits[:max_waits]
                    for cs in range(max_waits, len(waits), max_waits):
                        chunk = waits[cs:cs + max_waits]
                        d = nc.sync.nop(nofuse=True)
                        cur = nc.cur_bb.bb.instructions
                        assert cur[-1] is d.ins
                        cur.pop()
                        d.ins.engine = inst.engine
                        d.ins.sync_info = mybir.SyncInfo(on_wait=chunk, on_update=[])
                        insts.insert(i, d.ins)
                        i += 1
                i += 1


def build_program(mode=MODE):
    import concourse.bass as bass
    import concourse.mybir as mybir
    from concourse.masks import make_identity
    from concourse.tile import TileContext

    f32 = mybir.dt.float32
    io_dt = {"bf16": mybir.dt.bfloat16, "f32r": mybir.dt.float32r}[mode]
    snt_shape = [L, K] if mode == "bf16" else [128, 2 * K]

    nc = bass.Bass()
    x = nc.declare_dram_parameter("x", [ROWS, TPAD], io_dt, isOutput=False)
    snt_in = nc.declare_dram_parameter("snt", snt_shape, io_dt, isOutput=False)
    y = nc.declare_dram_parameter("y", [ROWS, OUT_COLS], f32, isOutput=True)

    def hankel_ap(b, col0, g_rows, width, n_chunk, chunk_step):
        """AP over x: dims (chunk, l, c) -> x[b, col0 + chunk*chunk_step + l + c]."""
        ap = x[b:b + 1, 0:width].copy()
        dims = [[1, g_rows], [1, width]]
        if n_chunk > 1:
            dims = [[chunk_step, n_chunk]] + dims
        ap.ap = mybir.VecI64Pair(dims)
        ap.offset = b * TPAD + col0
        return ap

    halves = [(0, HALF), (HALF, N - HALF)]   # (n0, n_windows)

    with TileContext(nc) as tc:
        with (
            tc.tile_pool(name="const", bufs=1) as const_pool,
            tc.tile_pool(name="hank", bufs=4) as hank_pool,
            tc.tile_pool(name="mrg", bufs=1) as mrg_pool,
        ):
            snt = const_pool.tile(snt_shape, io_dt)
            nc.scalar.dma_start(out=snt[:, :], in_=snt_in[:, :])
            ident = const_pool.tile([128, 128], f32)
            make_identity(nc, ident[:, :])
            # Result accumulator R[k, m*32 + b], m in (p1, p_mean, p2, dom).
            R = const_pool.tile([K, 128], f32)
            # Per-row sorted top-8 candidates: row b spans cols
            # [16b, 16b+16); span A top-8 first, span B top-8 second.
            CAND = const_pool.tile([K, 16 * ROWS], f32)

            with tc.tile_pool(name="psum", bufs=2, space="PSUM") as psum_pool:
                for b in range(ROWS):
                    eng = nc.sync if b % 2 == 0 else nc.scalar
                    if mode == "bf16":
                        h = hank_pool.tile([128, HW], io_dt, tag="hank")
                        eng.dma_start(out=h[:, :],
                                      in_=hankel_ap(b, 0, 128, HW, 1, 0))
                    else:
                        w = HALF + G
                        h = hank_pool.tile([128, w], io_dt, tag="hank")
                        for ci, ceng in ((0, nc.sync), (1, nc.scalar)):
                            ceng.dma_start(
                                out=h[ci * G:(ci + 1) * G, :],
                                in_=hankel_ap(b, ci * HALF, G, w, 1, 0))

                    for hi, (n0, nw) in enumerate(halves):
                        ps = psum_pool.tile([K, HALF], f32, tag="psum")
                        for j in range(0, HALF, WBLK):
                            if mode == "bf16":
                                nc.tensor.matmul(
                                    ps[:, j:j + WBLK], snt[:, :],
                                    h[:, n0 + j:n0 + j + WBLK],
                                    start=True, stop=True)
                            else:
                                r0 = hi * G
                                nc.tensor.matmul(
                                    ps[:, j:j + WBLK],
                                    snt[r0:r0 + G, 0:K],
                                    h[r0:r0 + G, j:j + WBLK],
                                    start=True, stop=False)
                                nc.tensor.matmul(
                                    ps[:, j:j + WBLK],
                                    snt[r0:r0 + G, K:2 * K],
                                    h[r0:r0 + G, j + G:j + G + WBLK],
                                    start=False, stop=True)
                        c0 = 16 * b + 8 * hi
                        nc.vector.max(out=CAND[:, c0:c0 + 8], in_=ps[:, :nw])

            # ---- batched finalize over all 32 rows ----------------------
            # a_i / b_i: [K, 32] strided views, element (k, b) = i-th
            # largest of row b's span A / span B.
            e = nc.gpsimd if MERGE_ENGINE == "gpsimd" else nc.vector
            mmin = mybir.AluOpType.min

            def a(i):
                return CAND[:, i::16]

            def bb(i):
                return CAND[:, 8 + i::16]

            _tmp_n = [0]

            def tmp():
                # unique tag per temp: each gets its own SBUF allocation
                # (same-tag tiles in a pool share a buffer ring)
                _tmp_n[0] += 1
                nm = f"mrgt{_tmp_n[0]}"
                return mrg_pool.tile([K, ROWS], f32, tag=nm, name=nm)

            p1 = R[:, 0:ROWS]
            pm = R[:, ROWS:2 * ROWS]
            p2 = R[:, 2 * ROWS:3 * ROWS]
            dom = R[:, 3 * ROWS:4 * ROWS]

            # t1
            e.tensor_max(p1, a(0), bb(0))
            # t2 = max(a1, b1, min(a0,b0))
            m01 = tmp()
            e.tensor_tensor(m01[:, :], a(0), bb(0), mmin)
            u = tmp()
            e.tensor_max(u[:, :], a(1), bb(1))
            e.tensor_max(p2, u[:, :], m01[:, :])
            # dominance = p1 - p2 (p1 >= p2 always)
            e.tensor_sub(dom, p1, p2)
            # t3 = max(a2, b2, min(a0,b1), min(a1,b0))
            v1, v2, v3 = tmp(), tmp(), tmp()
            e.tensor_max(v1[:, :], a(2), bb(2))
            e.tensor_tensor(v2[:, :], a(0), bb(1), mmin)
            e.tensor_tensor(v3[:, :], a(1), bb(0), mmin)
            e.tensor_max(v2[:, :], v2[:, :], v3[:, :])
            t3 = tmp()
            e.tensor_max(t3[:, :], v1[:, :], v2[:, :])
            # t4 = max(a3, b3, min(a0,b2), min(a1,b1), min(a2,b0))
            w1, w2, w3 = tmp(), tmp(), tmp()
            e.tensor_max(w1[:, :], a(3), bb(3))
            e.tensor_tensor(w2[:, :], a(0), bb(2), mmin)
            e.tensor_tensor(w3[:, :], a(1), bb(1), mmin)
            e.tensor_max(w2[:, :], w2[:, :], w3[:, :])
            e.tensor_tensor(w3[:, :], a(2), bb(0), mmin)
            e.tensor_max(w1[:, :], w1[:, :], w2[:, :])
            t4 = tmp()
            e.tensor_max(t4[:, :], w1[:, :], w3[:, :])
            # t5 = max(a4, b4, min(a0,b3), min(a1,b2), min(a2,b1), min(a3,b0))
            z1, z2, z3 = tmp(), tmp(), tmp()
            e.tensor_max(z1[:, :], a(4), bb(4))
            e.tensor_tensor(z2[:, :], a(0), bb(3), mmin)
            e.tensor_tensor(z3[:, :], a(1), bb(2), mmin)
            e.tensor_max(z2[:, :], z2[:, :], z3[:, :])
            e.tensor_tensor(z3[:, :], a(2), bb(1), mmin)
            e.tensor_max(z1[:, :], z1[:, :], z2[:, :])
            e.tensor_tensor(z2[:, :], a(3), bb(0), mmin)
            e.tensor_max(z3[:, :], z3[:, :], z2[:, :])
            t5 = tmp()
            e.tensor_max(t5[:, :], z1[:, :], z3[:, :])
            # p_mean = (t1 + t2 + t3 + t4 + t5) / 5
            s = tmp()
            e.tensor_add(s[:, :], p1, p2)
            e.tensor_add(s[:, :], s[:, :], t3[:, :])
            e.tensor_add(s[:, :], s[:, :], t4[:, :])
            e.tensor_add(s[:, :], s[:, :], t5[:, :])
            e.tensor_scalar_mul(pm, s[:, :], 1.0 / K_TOP)

            # Transpose R -> TR[m*32+b, k]; write y[b, m*128+k].
            with tc.tile_pool(name="tpsum", bufs=1, space="PSUM") as tpsum_pool:
                tr_ps = tpsum_pool.tile([128, 128], f32)
                nc.tensor.transpose(tr_ps[:, :], R[:, :], ident[:, :])
                tr = const_pool.tile([128, 128], f32)
                nc.scalar.copy(tr[:, :], tr_ps[:, :])
                for m in range(4):
                    nc.sync.dma_start(out=y[:, m * K:(m + 1) * K],
                                      in_=tr[m * ROWS:(m + 1) * ROWS, :])

    _split_excess_waits(nc, mybir)
    return nc


_CACHED = {}


def _get_program(mode=MODE):
    if mode not in _CACHED:
        _CACHED[mode] = build_program(mode)
    return _CACHED[mode]


def _prep_inputs(x, shapelets, mode=MODE):
    x = np.ascontiguousarray(x, dtype=np.float32)
    s = np.asarray(shapelets, dtype=np.float32)
    snt = np.ascontiguousarray((s - s.mean(axis=1, keepdims=True)).T)
    x = np.pad(x, ((0, 0), (0, TPAD - T)))
    if mode == "bf16":
        import ml_dtypes
        x = x.astype(ml_dtypes.bfloat16)
        snt = snt.astype(ml_dtypes.bfloat16)
    else:
        # [128, 2K]: G-row chunks side by side, tiled over both halves.
        snt = np.tile(np.concatenate([snt[0:G], snt[G:2 * G]], axis=1), (2, 1))
        snt = np.ascontiguousarray(snt)
    return x, snt


def run_sharded(x, shapelets, mode=MODE, trace=False, **kw):
    from concourse.bass_utils import run_bass_kernel_spmd

    nc = _get_program(mode)
    xp, snt = _prep_inputs(x, shapelets, mode)
    in_maps = [
        {"x": xp[c * ROWS:(c + 1) * ROWS], "snt": snt}
        for c in range(N_CORES)
    ]
    res = run_bass_kernel_spmd(nc, in_maps, list(range(N_CORES)), trace=trace, **kw)
    out = np.concatenate([res.results[c]["y"] for c in range(N_CORES)], axis=0)
    return out, res


def kernel(x, shapelets):
    out, _ = run_sharded(x, shapelets)
    return out


# revision 9
# speedup vs baseline: 1.2032x; 1.0059x over previous
"""ConvShapeletFilter kernel for Trainium2 (8 NeuronCores, data-parallel).

Math: reference computes, per batch row b and shapelet k,
    corr[b,n,k] = <x_win[b,n]-mean(x_win[b,n]), s[k]-mean(s[k])>
Since (s[k]-mean(s[k])) sums to zero over l, the window-mean term drops:
    corr[b,n,k] = sum_l x[b,n+l] * s_norm[k,l]
i.e. a plain cross-correlation with the mean-removed shapelet bank.
Outputs per (b,k): top-1, mean(top-5), top-2, relu(top1-top2) over n.

Device mapping (per core, 32 of 256 batch rows), mode "bf16" (default):
  - s_norm^T prepared host-side (tiny [128,128] op) and shipped bf16.
  - One Hankel/im2col tile per row, H[l, c] = x[b, l + c], [128, 4096]
    bf16, DMA'd straight from HBM with an overlapping access pattern.
    The AP's outermost dim is the 128-partition dim: the hardware DGE
    round-robins descriptors over the 16 DMA engines by outer-dim
    index, so this spreads the ~0.5 MB twice-per-row traffic evenly
    (a small outer dim pins everything to 2 engines - 3x slower
    end-to-end). Rows alternate between the SP and Activation HWDGE
    queues.
  - Per row, 2 PSUM spans of 2048 windows; 4 matmuls per span
    (full L=128 contraction, bf16 operands, fp32 PSUM).
  - DVE InstMax (sorted top-8 per partition) once per span -> CAND
    slots. DVE does nothing else in steady state: the per-row top-8
    merge + finalization is deferred and batched.
  - Batched finalize: with per-row sorted span lists a, b, the k-th
    largest of the union is max_{i+j=k-1} min(a_i, b_j). Computed for
    k=1..5 over all 32 rows at once via ~31 strided [128,32]
    elementwise ops, giving p1, p2, mean(top5), dominance directly.
    p1 >= p2 always, so relu(p1-p2) is a plain subtract.
  - One PE transpose of the [K, 128] result + 4 DMAs write y[32, 512].

Mode "f32r" (fallback): fp32 data bitcast float32r, G=64 filter split
with PSUM accumulation to halve DMA traffic; per-row finalize.
"""

import os
import sys

for _p in ("/opt/trn_rl_repo", os.path.expanduser("~/.axon_site/_ro/trn_rl_repo")):
    if os.path.isdir(_p) and _p not in sys.path:
        sys.path.insert(0, _p)

import numpy as np

MODE = os.environ.get("SHAPELET_MODE", "bf16")        # bf16 | f32r
MERGE_ENGINE = os.environ.get("SHAPELET_MERGE", "vector")  # vector | gpsimd

B, T = 256, 4096
L = 128
K = 128
K_TOP = 5
N = T - L + 1          # 3969 sliding windows
N_CORES = 8
ROWS = B // N_CORES    # 32 batch rows per core
WBLK = 512             # windows per matmul (1 PSUM bank fp32)
HALF = 2048            # windows per PSUM span (4 banks)
OUT_COLS = 4 * K       # p1 | p_mean | p2 | dominance
G = 64                 # f32r mode: filter-chunk size
TPAD = T + 2 * G       # padded x row length (4224)
HW = T                 # bf16 hankel tile width (cols 0..4095)


def _split_excess_waits(nc, mybir, max_waits=1):
    """Walrus CoreV3 codegen rejects >1 sync-wait on several instruction
    classes (CTRL/Drain, S3_LW/Matmult, ...). Hoist excess waits onto
    same-engine NoOps placed just before the offender."""
    for fn in nc.m.functions:
        for bb in fn.blocks:
            insts = bb.instructions
            i = 0
            while i < len(insts):
                inst = insts[i]
                si = inst.sync_info
                if (si is not None and si.on_wait
                        and len(si.on_wait) > max_waits):
                    waits = list(si.on_wait)
                    si.on_wait = waits[:max_waits]
                    for cs in range(max_waits, len(waits), max_waits):
                        chunk = waits[cs:cs + max_waits]
                        d = nc.sync.nop(nofuse=True)
                        cur = nc.cur_bb.bb.instructions
                        assert cur[-1] is d.ins
                        cur.pop()
                        d.ins.engine = inst.engine
                        d.ins.sync_info = mybir.SyncInfo(on_wait=chunk, on_update=[])
                        insts.insert(i, d.ins)
                        i += 1
                i += 1


def build_program(mode=MODE):
    import concourse.bass as bass
    import concourse.mybir as mybir
    from concourse.masks import make_identity
    from concourse.tile import TileContext

    f32 = mybir.dt.float32
    io_dt = {"bf16": mybir.dt.bfloat16, "f32r": mybir.dt.float32r}[mode]
    snt_shape = [L, K] if mode == "bf16" else [128, 2 * K]

    nc = bass.Bass()
    x = nc.declare_dram_parameter("x", [ROWS, TPAD], io_dt, isOutput=False)
    snt_in = nc.declare_dram_parameter("snt", snt_shape, io_dt, isOutput=False)
    y = nc.declare_dram_parameter("y", [ROWS, OUT_COLS], f32, isOutput=True)

    def hankel_ap(b, col0, g_rows, width, n_chunk, chunk_step):
        """AP over x: dims (chunk, l, c) -> x[b, col0 + chunk*chunk_step + l + c]."""
        ap = x[b:b + 1, 0:width].copy()
        dims = [[1, g_rows], [1, width]]
        if n_chunk > 1:
            dims = [[chunk_step, n_chunk]] + dims
        ap.ap = mybir.VecI64Pair(dims)
        ap.offset = b * TPAD + col0
        return ap

    halves = [(0, HALF), (HALF, N - HALF)]   # (n0, n_windows)

    with TileContext(nc) as tc:
        with (
            tc.tile_pool(name="const", bufs=1) as const_pool,
            tc.tile_pool(name="hank", bufs=4) as hank_pool,
            tc.tile_pool(name="mrg", bufs=1) as mrg_pool,
        ):
            snt = const_pool.tile(snt_shape, io_dt)
            nc.scalar.dma_start(out=snt[:, :], in_=snt_in[:, :])
            ident = const_pool.tile([128, 128], f32)
            make_identity(nc, ident[:, :])
            # Result accumulator R[k, m*32 + b], m in (p1, p_mean, p2, dom).
            R = const_pool.tile([K, 128], f32)
            # Per-row sorted top-8 candidates: row b spans cols
            # [16b, 16b+16); span A top-8 first, span B top-8 second.
            CAND = const_pool.tile([K, 16 * ROWS], f32)

            with tc.tile_pool(name="psum", bufs=2, space="PSUM") as psum_pool:
                for b in range(ROWS):
                    eng = nc.sync if b % 2 == 0 else nc.scalar
                    if mode == "bf16":
                        h = hank_pool.tile([128, HW], io_dt, tag="hank")
                        eng.dma_start(out=h[:, :],
                                      in_=hankel_ap(b, 0, 128, HW, 1, 0))
                    else:
                        w = HALF + G
                        h = hank_pool.tile([128, w], io_dt, tag="hank")
                        for ci, ceng in ((0, nc.sync), (1, nc.scalar)):
                            ceng.dma_start(
                                out=h[ci * G:(ci + 1) * G, :],
                                in_=hankel_ap(b, ci * HALF, G, w, 1, 0))

                    for hi, (n0, nw) in enumerate(halves):
                        ps = psum_pool.tile([K, HALF], f32, tag="psum")
                        for j in range(0, HALF, WBLK):
                            if mode == "bf16":
                                nc.tensor.matmul(
                                    ps[:, j:j + WBLK], snt[:, :],
                                    h[:, n0 + j:n0 + j + WBLK],
                                    start=True, stop=True)
                            else:
                                r0 = hi * G
                                nc.tensor.matmul(
                                    ps[:, j:j + WBLK],
                                    snt[r0:r0 + G, 0:K],
                                    h[r0:r0 + G, j:j + WBLK],
                                    start=True, stop=False)
                                nc.tensor.matmul(
                                    ps[:, j:j + WBLK],
                                    snt[r0:r0 + G, K:2 * K],
                                    h[r0:r0 + G, j + G:j + G + WBLK],
                                    start=False, stop=True)
                        c0 = 16 * b + 8 * hi
                        nc.vector.max(out=CAND[:, c0:c0 + 8], in_=ps[:, :nw])

            # ---- batched finalize over all 32 rows ----------------------
            # a_i / b_i: [K, 32] strided views, element (k, b) = i-th
            # largest of row b's span A / span B.
            e = nc.gpsimd if MERGE_ENGINE == "gpsimd" else nc.vector
            mmin = mybir.AluOpType.min

            def a(i):
                return CAND[:, i::16]

            def bb(i):
                return CAND[:, 8 + i::16]

            _tmp_n = [0]

            def tmp():
                # unique tag per temp: each gets its own SBUF allocation
                # (same-tag tiles in a pool share a buffer ring)
                _tmp_n[0] += 1
                nm = f"mrgt{_tmp_n[0]}"
                return mrg_pool.tile([K, ROWS], f32, tag=nm, name=nm)

            p1 = R[:, 0:ROWS]
            pm = R[:, ROWS:2 * ROWS]
            p2 = R[:, 2 * ROWS:3 * ROWS]
            dom = R[:, 3 * ROWS:4 * ROWS]

            # t1
            e.tensor_max(p1, a(0), bb(0))
            # t2 = max(a1, b1, min(a0,b0))
            m01 = tmp()
            e.tensor_tensor(m01[:, :], a(0), bb(0), mmin)
            u = tmp()
            e.tensor_max(u[:, :], a(1), bb(1))
            e.tensor_max(p2, u[:, :], m01[:, :])
            # dominance = p1 - p2 (p1 >= p2 always)
            e.tensor_sub(dom, p1, p2)
            # t3 = max(a2, b2, min(a0,b1), min(a1,b0))
            v1, v2, v3 = tmp(), tmp(), tmp()
            e.tensor_max(v1[:, :], a(2), bb(2))
            e.tensor_tensor(v2[:, :], a(0), bb(1), mmin)
            e.tensor_tensor(v3[:, :], a(1), bb(0), mmin)
            e.tensor_max(v2[:, :], v2[:, :], v3[:, :])
            t3 = tmp()
            e.tensor_max(t3[:, :], v1[:, :], v2[:, :])
            # t4 = max(a3, b3, min(a0,b2), min(a1,b1), min(a2,b0))
            w1, w2, w3 = tmp(), tmp(), tmp()
            e.tensor_max(w1[:, :], a(3), bb(3))
            e.tensor_tensor(w2[:, :], a(0), bb(2), mmin)
            e.tensor_tensor(w3[:, :], a(1), bb(1), mmin)
            e.tensor_max(w2[:, :], w2[:, :], w3[:, :])
            e.tensor_tensor(w3[:, :], a(2), bb(0), mmin)
            e.tensor_max(w1[:, :], w1[:, :], w2[:, :])
            t4 = tmp()
            e.tensor_max(t4[:, :], w1[:, :], w3[:, :])
            # t5 = max(a4, b4, min(a0,b3), min(a1,b2), min(a2,b1), min(a3,b0))
            z1, z2, z3 = tmp(), tmp(), tmp()
            e.tensor_max(z1[:, :], a(4), bb(4))
            e.tensor_tensor(z2[:, :], a(0), bb(3), mmin)
            e.tensor_tensor(z3[:, :], a(1), bb(2), mmin)
            e.tensor_max(z2[:, :], z2[:, :], z3[:, :])
            e.tensor_tensor(z3[:, :], a(2), bb(1), mmin)
            e.tensor_max(z1[:, :], z1[:, :], z2[:, :])
            e.tensor_tensor(z2[:, :], a(3), bb(0), mmin)
            e.tensor_max(z3[:, :], z3[:, :], z2[:, :])
            t5 = tmp()
            e.tensor_max(t5[:, :], z1[:, :], z3[:, :])
            # p_mean = (t1 + t2 + t3 + t4 + t5) / 5
            s = tmp()
            e.tensor_add(s[:, :], p1, p2)
            e.tensor_add(s[:, :], s[:, :], t3[:, :])
            e.tensor_add(s[:, :], s[:, :], t4[:, :])
            e.tensor_add(s[:, :], s[:, :], t5[:, :])
            e.tensor_scalar_mul(pm, s[:, :], 1.0 / K_TOP)

            # Transpose R -> TR[m*32+b, k]; write y[b, m*128+k].
            with tc.tile_pool(name="tpsum", bufs=1, space="PSUM") as tpsum_pool:
                tr_ps = tpsum_pool.tile([128, 128], f32)
                nc.tensor.transpose(tr_ps[:, :], R[:, :], ident[:, :])
                tr = const_pool.tile([128, 128], f32)
                nc.scalar.copy(tr[:, :], tr_ps[:, :])
                for m in range(4):
                    oeng = nc.sync if m % 2 == 0 else nc.scalar
                    oeng.dma_start(out=y[:, m * K:(m + 1) * K],
                                   in_=tr[m * ROWS:(m + 1) * ROWS, :])

    _split_excess_waits(nc, mybir)
    return nc


_CACHED = {}


def _get_program(mode=MODE):
    if mode not in _CACHED:
        _CACHED[mode] = build_program(mode)
    return _CACHED[mode]


def _prep_inputs(x, shapelets, mode=MODE):
    x = np.ascontiguousarray(x, dtype=np.float32)
    s = np.asarray(shapelets, dtype=np.float32)
    snt = np.ascontiguousarray((s - s.mean(axis=1, keepdims=True)).T)
    x = np.pad(x, ((0, 0), (0, TPAD - T)))
    if mode == "bf16":
        import ml_dtypes
        x = x.astype(ml_dtypes.bfloat16)
        snt = snt.astype(ml_dtypes.bfloat16)
    else:
        # [128, 2K]: G-row chunks side by side, tiled over both halves.
        snt = np.tile(np.concatenate([snt[0:G], snt[G:2 * G]], axis=1), (2, 1))
        snt = np.ascontiguousarray(snt)
    return x, snt


def run_sharded(x, shapelets, mode=MODE, trace=False, **kw):
    from concourse.bass_utils import run_bass_kernel_spmd

    nc = _get_program(mode)
    xp, snt = _prep_inputs(x, shapelets, mode)
    in_maps = [
        {"x": xp[c * ROWS:(c + 1) * ROWS], "snt": snt}
        for c in range(N_CORES)
    ]
    res = run_bass_kernel_spmd(nc, in_maps, list(range(N_CORES)), trace=trace, **kw)
    out = np.concatenate([res.results[c]["y"] for c in range(N_CORES)], axis=0)
    return out, res


def kernel(x, shapelets):
    out, _ = run_sharded(x, shapelets)
    return out


# revision 10
# speedup vs baseline: 1.2074x; 1.0035x over previous
"""ConvShapeletFilter kernel for Trainium2 (8 NeuronCores, data-parallel).

Math: reference computes, per batch row b and shapelet k,
    corr[b,n,k] = <x_win[b,n]-mean(x_win[b,n]), s[k]-mean(s[k])>
Since (s[k]-mean(s[k])) sums to zero over l, the window-mean term drops:
    corr[b,n,k] = sum_l x[b,n+l] * s_norm[k,l]
i.e. a plain cross-correlation with the mean-removed shapelet bank.
Outputs per (b,k): top-1, mean(top-5), top-2, relu(top1-top2) over n.

Device mapping (per core, 32 of 256 batch rows), mode "bf16" (default):
  - s_norm^T prepared host-side (tiny [128,128] op) and shipped bf16.
  - One Hankel/im2col tile per row, H[l, c] = x[b, l + c], [128, 4096]
    bf16, DMA'd straight from HBM with an overlapping access pattern.
    The AP's outermost dim is the 128-partition dim: the hardware DGE
    round-robins descriptors over the 16 DMA engines by outer-dim
    index, so this spreads the ~1 MB once-per-row traffic evenly
    (a small outer dim pins everything to 2 engines - 3x slower
    end-to-end). Rows alternate between the SP and Activation HWDGE
    queues.
  - Per row, 2 PSUM spans of 2048 windows; 4 matmuls per span
    (full L=128 contraction, bf16 operands, fp32 PSUM).
  - DVE InstMax (sorted top-8 per partition) once per span -> CAND
    slots. DVE does nothing else in steady state: the per-row top-8
    merge + finalization is deferred and batched.
  - Batched finalize: with per-row sorted span lists a, b, the k-th
    largest of the union is max_{i+j=k-1} min(a_i, b_j). Computed for
    k=1..5 over all 32 rows at once via ~31 strided [128,32]
    elementwise ops, giving p1, p2, mean(top5), dominance directly.
    p1 >= p2 always, so relu(p1-p2) is a plain subtract.
  - One PE transpose of the [K, 128] result + 4 DMAs write y[32, 512].

Mode "f32r" (fallback): fp32 data bitcast float32r, G=64 filter split
with PSUM accumulation to halve DMA traffic; per-row finalize.
"""

import os
import sys

for _p in ("/opt/trn_rl_repo", os.path.expanduser("~/.axon_site/_ro/trn_rl_repo")):
    if os.path.isdir(_p) and _p not in sys.path:
        sys.path.insert(0, _p)

import numpy as np

MODE = os.environ.get("SHAPELET_MODE", "bf16")        # bf16 | f32r
MERGE_ENGINE = os.environ.get("SHAPELET_MERGE", "vector")  # vector | gpsimd

B, T = 256, 4096
L = 128
K = 128
K_TOP = 5
N = T - L + 1          # 3969 sliding windows
N_CORES = 8
ROWS = B // N_CORES    # 32 batch rows per core
WBLK = 512             # windows per matmul (1 PSUM bank fp32)
HALF = 2048            # windows per PSUM span (4 banks)
OUT_COLS = 4 * K       # p1 | p_mean | p2 | dominance
G = 64                 # f32r mode: filter-chunk size
TPAD = T + 2 * G       # padded x row length (4224)
HW = T                 # bf16 hankel tile width (cols 0..4095)


def _split_excess_waits(nc, mybir, max_waits=1):
    """Walrus CoreV3 codegen rejects >1 sync-wait on several instruction
    classes (CTRL/Drain, S3_LW/Matmult, ...). Hoist excess waits onto
    same-engine NoOps placed just before the offender."""
    for fn in nc.m.functions:
        for bb in fn.blocks:
            insts = bb.instructions
            i = 0
            while i < len(insts):
                inst = insts[i]
                si = inst.sync_info
                if (si is not None and si.on_wait
                        and len(si.on_wait) > max_waits):
                    waits = list(si.on_wait)
                    si.on_wait = waits[:max_waits]
                    for cs in range(max_waits, len(waits), max_waits):
                        chunk = waits[cs:cs + max_waits]
                        d = nc.sync.nop(nofuse=True)
                        cur = nc.cur_bb.bb.instructions
                        assert cur[-1] is d.ins
                        cur.pop()
                        d.ins.engine = inst.engine
                        d.ins.sync_info = mybir.SyncInfo(on_wait=chunk, on_update=[])
                        insts.insert(i, d.ins)
                        i += 1
                i += 1


def build_program(mode=MODE):
    import concourse.bass as bass
    import concourse.mybir as mybir
    from concourse.masks import make_identity
    from concourse.tile import TileContext

    f32 = mybir.dt.float32
    io_dt = {"bf16": mybir.dt.bfloat16, "f32r": mybir.dt.float32r}[mode]
    snt_shape = [L, K] if mode == "bf16" else [128, 2 * K]

    nc = bass.Bass()
    x = nc.declare_dram_parameter("x", [ROWS, TPAD], io_dt, isOutput=False)
    snt_in = nc.declare_dram_parameter("snt", snt_shape, io_dt, isOutput=False)
    y = nc.declare_dram_parameter("y", [ROWS, OUT_COLS], f32, isOutput=True)

    def hankel_ap(b, col0, g_rows, width, n_chunk, chunk_step):
        """AP over x: dims (chunk, l, c) -> x[b, col0 + chunk*chunk_step + l + c]."""
        ap = x[b:b + 1, 0:width].copy()
        dims = [[1, g_rows], [1, width]]
        if n_chunk > 1:
            dims = [[chunk_step, n_chunk]] + dims
        ap.ap = mybir.VecI64Pair(dims)
        ap.offset = b * TPAD + col0
        return ap

    halves = [(0, HALF), (HALF, N - HALF)]   # (n0, n_windows)

    with TileContext(nc) as tc:
        with (
            tc.tile_pool(name="const", bufs=1) as const_pool,
            tc.tile_pool(name="hank", bufs=4) as hank_pool,
            tc.tile_pool(name="mrg", bufs=1) as mrg_pool,
        ):
            snt = const_pool.tile(snt_shape, io_dt)
            nc.scalar.dma_start(out=snt[:, :], in_=snt_in[:, :])
            ident = const_pool.tile([128, 128], f32)
            make_identity(nc, ident[:, :])
            # Result accumulator R[k, m*32 + b], m in (p1, p_mean, p2, dom).
            R = const_pool.tile([K, 128], f32)
            # Per-row sorted top-8 candidates: row b spans cols
            # [16b, 16b+16); span A top-8 first, span B top-8 second.
            CAND = const_pool.tile([K, 16 * ROWS], f32)

            with tc.tile_pool(name="psum", bufs=2, space="PSUM") as psum_pool:
                for b in range(ROWS):
                    eng = nc.sync if b % 2 == 0 else nc.scalar
                    if mode == "bf16":
                        h = hank_pool.tile([128, HW], io_dt, tag="hank")
                        eng.dma_start(out=h[:, :],
                                      in_=hankel_ap(b, 0, 128, HW, 1, 0))
                    else:
                        w = HALF + G
                        h = hank_pool.tile([128, w], io_dt, tag="hank")
                        for ci, ceng in ((0, nc.sync), (1, nc.scalar)):
                            ceng.dma_start(
                                out=h[ci * G:(ci + 1) * G, :],
                                in_=hankel_ap(b, ci * HALF, G, w, 1, 0))

                    for hi, (n0, nw) in enumerate(halves):
                        ps = psum_pool.tile([K, HALF], f32, tag="psum")
                        for j in range(0, HALF, WBLK):
                            if mode == "bf16":
                                nc.tensor.matmul(
                                    ps[:, j:j + WBLK], snt[:, :],
                                    h[:, n0 + j:n0 + j + WBLK],
                                    start=True, stop=True)
                            else:
                                r0 = hi * G
                                nc.tensor.matmul(
                                    ps[:, j:j + WBLK],
                                    snt[r0:r0 + G, 0:K],
                                    h[r0:r0 + G, j:j + WBLK],
                                    start=True, stop=False)
                                nc.tensor.matmul(
                                    ps[:, j:j + WBLK],
                                    snt[r0:r0 + G, K:2 * K],
                                    h[r0:r0 + G, j + G:j + G + WBLK],
                                    start=False, stop=True)
                        c0 = 16 * b + 8 * hi
                        nc.vector.max(out=CAND[:, c0:c0 + 8], in_=ps[:, :nw])

            # ---- batched finalize over all 32 rows ----------------------
            # a_i / b_i: [K, 32] strided views, element (k, b) = i-th
            # largest of row b's span A / span B.
            e = nc.gpsimd if MERGE_ENGINE == "gpsimd" else nc.vector
            mmin = mybir.AluOpType.min

            def a(i):
                return CAND[:, i::16]

            def bb(i):
                return CAND[:, 8 + i::16]

            _tmp_n = [0]

            def tmp():
                # unique tag per temp: each gets its own SBUF allocation
                # (same-tag tiles in a pool share a buffer ring)
                _tmp_n[0] += 1
                nm = f"mrgt{_tmp_n[0]}"
                return mrg_pool.tile([K, ROWS], f32, tag=nm, name=nm)

            p1 = R[:, 0:ROWS]
            pm = R[:, ROWS:2 * ROWS]
            p2 = R[:, 2 * ROWS:3 * ROWS]
            dom = R[:, 3 * ROWS:4 * ROWS]

            # t1
            e.tensor_max(p1, a(0), bb(0))
            # t2 = max(a1, b1, min(a0,b0))
            m01 = tmp()
            e.tensor_tensor(m01[:, :], a(0), bb(0), mmin)
            u = tmp()
            e.tensor_max(u[:, :], a(1), bb(1))
            e.tensor_max(p2, u[:, :], m01[:, :])
            # dominance = p1 - p2 (p1 >= p2 always)
            e.tensor_sub(dom, p1, p2)
            # t3 = max(a2, b2, min(a0,b1), min(a1,b0))
            v1, v2, v3 = tmp(), tmp(), tmp()
            e.tensor_max(v1[:, :], a(2), bb(2))
            e.tensor_tensor(v2[:, :], a(0), bb(1), mmin)
            e.tensor_tensor(v3[:, :], a(1), bb(0), mmin)
            e.tensor_max(v2[:, :], v2[:, :], v3[:, :])
            t3 = tmp()
            e.tensor_max(t3[:, :], v1[:, :], v2[:, :])
            # t4 = max(a3, b3, min(a0,b2), min(a1,b1), min(a2,b0))
            w1, w2, w3 = tmp(), tmp(), tmp()
            e.tensor_max(w1[:, :], a(3), bb(3))
            e.tensor_tensor(w2[:, :], a(0), bb(2), mmin)
            e.tensor_tensor(w3[:, :], a(1), bb(1), mmin)
            e.tensor_max(w2[:, :], w2[:, :], w3[:, :])
            e.tensor_tensor(w3[:, :], a(2), bb(0), mmin)
            e.tensor_max(w1[:, :], w1[:, :], w2[:, :])
            t4 = tmp()
            e.tensor_max(t4[:, :], w1[:, :], w3[:, :])
            # t5 = max(a4, b4, min(a0,b3), min(a1,b2), min(a2,b1), min(a3,b0))
            z1, z2, z3 = tmp(), tmp(), tmp()
            e.tensor_max(z1[:, :], a(4), bb(4))
            e.tensor_tensor(z2[:, :], a(0), bb(3), mmin)
            e.tensor_tensor(z3[:, :], a(1), bb(2), mmin)
            e.tensor_max(z2[:, :], z2[:, :], z3[:, :])
            e.tensor_tensor(z3[:, :], a(2), bb(1), mmin)
            e.tensor_max(z1[:, :], z1[:, :], z2[:, :])
            e.tensor_tensor(z2[:, :], a(3), bb(0), mmin)
            e.tensor_max(z3[:, :], z3[:, :], z2[:, :])
            t5 = tmp()
            e.tensor_max(t5[:, :], z1[:, :], z3[:, :])
            # p_mean = (t1 + t2 + t3 + t4 + t5) / 5
            s = tmp()
            e.tensor_add(s[:, :], p1, p2)
            e.tensor_add(s[:, :], s[:, :], t3[:, :])
            e.tensor_add(s[:, :], s[:, :], t4[:, :])
            e.tensor_add(s[:, :], s[:, :], t5[:, :])
            e.tensor_scalar_mul(pm, s[:, :], 1.0 / K_TOP)

            # Transpose R -> TR[m*32+b, k]; write y[b, m*128+k].
            with tc.tile_pool(name="tpsum", bufs=1, space="PSUM") as tpsum_pool:
                tr_ps = tpsum_pool.tile([128, 128], f32)
                nc.tensor.transpose(tr_ps[:, :], R[:, :], ident[:, :])
                tr = const_pool.tile([128, 128], f32)
                nc.scalar.copy(tr[:, :], tr_ps[:, :])
                for m in range(4):
                    oeng = nc.sync if m % 2 == 0 else nc.scalar
                    oeng.dma_start(out=y[:, m * K:(m + 1) * K],
                                   in_=tr[m * ROWS:(m + 1) * ROWS, :])

    _split_excess_waits(nc, mybir)
    return nc


_CACHED = {}


def _get_program(mode=MODE):
    if mode not in _CACHED:
        _CACHED[mode] = build_program(mode)
    return _CACHED[mode]


def _prep_inputs(x, shapelets, mode=MODE):
    x = np.ascontiguousarray(x, dtype=np.float32)
    s = np.asarray(shapelets, dtype=np.float32)
    snt = np.ascontiguousarray((s - s.mean(axis=1, keepdims=True)).T)
    x = np.pad(x, ((0, 0), (0, TPAD - T)))
    if mode == "bf16":
        import ml_dtypes
        x = x.astype(ml_dtypes.bfloat16)
        snt = snt.astype(ml_dtypes.bfloat16)
    else:
        # [128, 2K]: G-row chunks side by side, tiled over both halves.
        snt = np.tile(np.concatenate([snt[0:G], snt[G:2 * G]], axis=1), (2, 1))
        snt = np.ascontiguousarray(snt)
    return x, snt


def run_sharded(x, shapelets, mode=MODE, trace=False, **kw):
    from concourse.bass_utils import run_bass_kernel_spmd

    nc = _get_program(mode)
    xp, snt = _prep_inputs(x, shapelets, mode)
    in_maps = [
        {"x": xp[c * ROWS:(c + 1) * ROWS], "snt": snt}
        for c in range(N_CORES)
    ]
    res = run_bass_kernel_spmd(nc, in_maps, list(range(N_CORES)), trace=trace, **kw)
    out = np.concatenate([res.results[c]["y"] for c in range(N_CORES)], axis=0)
    return out, res


def kernel(x, shapelets):
    out, _ = run_sharded(x, shapelets)
    return out
